# revision 4
# baseline (speedup 1.0000x reference)
"""CLIP text embeddings (token-id gather + position-embedding add) on 8
Trainium2 NeuronCores via a raw Bass kernel.

Sharding: the 768-wide d_model axis is split 8 ways (96 columns per
core); the 77 token ids are replicated. Each core holds the full
49408-row token table restricted to its 96 columns, so every output
element is owned by exactly one core and no collective is needed (a
Megatron-style vocab-parallel split would pay a 236 KB AllReduce that
costs more than this whole kernel).

Per-core device program ("qord": 2 DMA-completion semaphores total):

  1. sync engine (HWDGE): DMA 1 loads a packed [77, 97] f32 input
     (pos-embedding slice in cols 0..95, token ids bitcast to f32 in
     col 96, one id per SBUF partition); DMA 2 copies the pos slice
     HBM->HBM straight into the output buffer.
  2. gpsimd: one indirect DMA gathers the 77 requested token rows from
     HBM into SBUF (bypass), then -- with NO completion wait -- a store
     with accum_op=add is enqueued on the SAME SWDGE queue
     (qPoolDynamic). Per-queue in-order descriptor execution makes the
     gather->store data dependency safe without a semaphore, and the
     CCE read-modify-write adds the gathered rows onto the pos values
     already sitting in the output.

Eliminating the gather-completion semaphore (~0.9 us DMA-sem
propagation) plus the separate vector add is worth ~1.8 us over the
3-stage version: ~5.6 us vs ~7.4 us steady-state.

The host concatenates the 8 column shards into [1, 77, 768].

Raw Bass (no TileContext) keeps the kernel tail free of Tile's
drain + EVSEM barrier (~9-17 us).
"""

import sys

sys.path.insert(0, "/opt/trn_rl_repo")

import numpy as np

VOCAB = 49408
D_MODEL = 768
SEQ_LEN = 77
N_CORES = 8
D_SHARD = D_MODEL // N_CORES  # 96
P = 128  # SBUF partitions

_cache = {}


def build_nc(n=1):
    """The device program, unrolled n times serialized by semaphores
    (n=1 is the real kernel; n>1 is used by test.py's delta timing)."""
    from concourse import bass, mybir

    D = D_SHARD
    nc = bass.Bass(num_devices=N_CORES)
    inp_d = nc.dram_tensor(
        "inp", [P, D + 1], mybir.dt.float32, kind="ExternalInput"
    )
    tok_d = nc.dram_tensor(
        "tok_w", [VOCAB, D], mybir.dt.float32, kind="ExternalInput"
    )
    out_d = nc.dram_tensor(
        "out", [SEQ_LEN, D], mybir.dt.float32, kind="ExternalOutput"
    )

    with (
        nc.sbuf_tensor([P, D + 1], mybir.dt.float32) as acc_t,
        nc.sbuf_tensor([P, D], mybir.dt.float32) as tok_t,
        nc.semaphore() as s_in,
        nc.semaphore() as s_pos,
        nc.semaphore() as s_tok,
        nc.semaphore() as s_out,
        nc.Block() as block,
    ):

        @block.sync
        def _(sync):
            for i in range(n):
                sync.wait_ge(s_out, 16 * i)
                sync.dma_start(
                    out=acc_t[:SEQ_LEN, :], in_=inp_d[:SEQ_LEN, :]
                ).then_inc(s_in, 16)
                sync.dma_start(
                    out=out_d[:], in_=inp_d[:SEQ_LEN, :D]
                ).then_inc(s_pos, 16)
            sync.wait_ge(s_out, 16 * n)

        @block.gpsimd
        def _(gp):
            for i in range(n):
                gp.wait_ge(s_in, 16 * (i + 1))
                nc.gpsimd.indirect_dma_start(
                    out=tok_t[:SEQ_LEN, :],
                    out_offset=None,
                    in_=tok_d[:],
                    in_offset=bass.IndirectOffsetOnAxis(
                        ap=acc_t[:SEQ_LEN, D : D + 1].bitcast(mybir.dt.int32),
                        axis=0,
                    ),
                    compute_op=mybir.AluOpType.bypass,
                ).then_inc(s_tok, 16)
                gp.wait_ge(s_pos, 16 * (i + 1))
                gp.dma_start(
                    out=out_d[:],
                    in_=tok_t[:SEQ_LEN, :],
                    accum_op=mybir.AluOpType.add,
                    single_packet=True,
                ).then_inc(s_out, 16)
            gp.wait_ge(s_out, 16 * n)

    return nc


def _get_runner():
    """Build the Bass module once and cache a jitted SPMD callable so
    repeat kernel() calls skip retracing/compilation."""
    if "runner" in _cache:
        return _cache["runner"]

    import jax
    from jax.sharding import Mesh, PartitionSpec
    from jax.experimental.shard_map import shard_map
    from concourse import bass2jax, mybir

    bass2jax.install_neuronx_cc_hook()
    nc = build_nc(1)
    partition_name = nc.partition_id_tensor.name if nc.partition_id_tensor else None

    in_names, out_names, out_avals, zero_outs = [], [], [], []
    for alloc in nc.m.functions[0].allocations:
        if not isinstance(alloc, mybir.MemoryLocationSet):
            continue
        name = alloc.memorylocations[0].name
        if alloc.kind == "ExternalInput":
            if name != partition_name:
                in_names.append(name)
        elif alloc.kind == "ExternalOutput":
            out_names.append(name)
            shape = tuple(alloc.tensor_shape)
            dtype = mybir.dt.np(alloc.dtype)
            out_avals.append(jax.core.ShapedArray(shape, dtype))
            zero_outs.append(np.zeros(shape, dtype))

    n_params = len(in_names)
    n_outs = len(out_avals)
    all_in_names = list(in_names) + list(out_names)
    if partition_name is not None:
        all_in_names.append(partition_name)

    def _body(*args):
        operands = list(args)
        if partition_name is not None:
            operands.append(bass2jax.partition_id_tensor())
        outs = bass2jax._bass_exec_p.bind(
            *operands,
            out_avals=tuple(out_avals),
            in_names=tuple(all_in_names),
            out_names=tuple(out_names),
            lowering_input_output_aliases=(),
            sim_require_finite=True,
            sim_require_nnan=True,
            nc=nc,
        )
        return tuple(outs)

    devices = jax.devices()[:N_CORES]
    assert len(devices) == N_CORES, f"need {N_CORES} devices, got {len(devices)}"
    mesh = Mesh(np.asarray(devices), ("core",))
    sharded = jax.jit(
        shard_map(
            _body,
            mesh=mesh,
            in_specs=(PartitionSpec("core"),) * (n_params + n_outs),
            out_specs=(PartitionSpec("core"),) * n_outs,
            check_rep=False,
        ),
        donate_argnums=tuple(range(n_params, n_params + n_outs)),
        keep_unused=True,
    )

    def run(concat_in_by_name):
        concat_in = [concat_in_by_name[name] for name in in_names]
        concat_zeros = [
            np.zeros((N_CORES * z.shape[0], *z.shape[1:]), z.dtype)
            for z in zero_outs
        ]
        out_arrs = sharded(*concat_in, *concat_zeros)
        return [
            {
                name: np.asarray(out_arrs[i]).reshape(
                    N_CORES, *out_avals[i].shape
                )[c]
                for i, name in enumerate(out_names)
            }
            for c in range(N_CORES)
        ]

    runner = {
        "run": run,
        "mesh": mesh,
        "PartitionSpec": PartitionSpec,
        "device_put": jax.device_put,
    }
    _cache["runner"] = runner
    return runner


def _tok_shards_device(tok, runner):
    """Column-shard the token table and push it to the 8 devices once;
    reuse across calls when the same table is passed again."""
    import jax

    sample = tok[::1997, ::13]
    key = (
        id(tok),
        tok.shape,
        str(tok.dtype),
        hash(sample.tobytes()),
        hash(tok[0, :8].tobytes()),
        hash(tok[-1, -8:].tobytes()),
    )
    hit = _cache.get("tok_dev")
    if hit is not None and hit[0] == key:
        return hit[1]
    # [V, 768] -> [8, V, 96] -> concat layout [8*V, 96], one pass
    shards = np.ascontiguousarray(
        tok.reshape(VOCAB, N_CORES, D_SHARD).swapaxes(0, 1)
    ).reshape(N_CORES * VOCAB, D_SHARD)
    sh = jax.sharding.NamedSharding(
        runner["mesh"], runner["PartitionSpec"]("core")
    )
    dev = runner["device_put"](shards, sh)
    jax.block_until_ready(dev)
    _cache["tok_dev"] = (key, dev)
    return dev


def kernel(
    input_ids: np.ndarray,
    position_ids: np.ndarray,
    token_embedding_weight: np.ndarray,
    position_embedding_weight: np.ndarray,
) -> np.ndarray:
    runner = _get_runner()

    ids = np.asarray(input_ids).astype(np.int32, copy=False)
    assert ids.shape == (SEQ_LEN,), ids.shape
    tok = np.ascontiguousarray(
        np.asarray(token_embedding_weight, dtype=np.float32)
    )
    pos_table = np.asarray(position_embedding_weight, dtype=np.float32)
    pids = np.asarray(position_ids).astype(np.int64, copy=False)
    if np.array_equal(pids, np.arange(SEQ_LEN)):
        pos = pos_table
    else:
        # CLIP always uses arange positions; reorder the tiny replicated
        # table during input prep if a caller ever passes something else.
        pos = pos_table[pids]

    # packed per-core input: [8, 128, 97] -> concat layout [8*128, 97]
    packed = np.zeros((N_CORES, P, D_SHARD + 1), np.float32)
    packed[:, :SEQ_LEN, :D_SHARD] = (
        pos.reshape(SEQ_LEN, N_CORES, D_SHARD).swapaxes(0, 1)
    )
    packed[:, :SEQ_LEN, D_SHARD] = ids.view(np.float32)[None, :]

    concat_in = {
        "inp": packed.reshape(N_CORES * P, D_SHARD + 1),
        "tok_w": _tok_shards_device(tok, runner),
    }
    results = runner["run"](concat_in)
    out = np.concatenate([results[c]["out"] for c in range(N_CORES)], axis=1)
    return out[None, :, :]


# revision 6
# speedup vs baseline: 1.0894x; 1.0894x over previous
"""CLIP text embeddings (token-id gather + position-embedding add) on 8
Trainium2 NeuronCores via a raw Bass kernel.

Sharding: the 768-wide d_model axis is split 8 ways (96 columns per
core); the 77 token ids are replicated. Each core holds the full
49408-row token table restricted to its 96 columns, so every output
element is owned by exactly one core and no collective is needed (a
Megatron-style vocab-parallel split would pay a 236 KB AllReduce that
costs more than this whole kernel).

Per-core device program ("qord": 2 DMA-completion semaphores total):

  1. sync engine (HWDGE): DMA 1 loads a packed [77, 97] f32 input
     (pos-embedding slice in cols 0..95, token ids bitcast to f32 in
     col 96, one id per SBUF partition); DMA 2 copies the pos slice
     HBM->HBM straight into the output buffer.
  2. gpsimd: one indirect DMA gathers the 77 requested token rows from
     HBM into SBUF (bypass), then -- with NO completion wait -- a store
     with accum_op=add is enqueued on the SAME SWDGE queue
     (qPoolDynamic). Per-queue in-order descriptor execution makes the
     gather->store data dependency safe without a semaphore, and the
     CCE read-modify-write adds the gathered rows onto the pos values
     already sitting in the output.

Eliminating the gather-completion semaphore (~0.9 us DMA-sem
propagation) plus the separate vector add is worth ~1.8 us over the
3-stage version: ~5.6 us vs ~7.4 us steady-state.

The host concatenates the 8 column shards into [1, 77, 768].

Raw Bass (no TileContext) keeps the kernel tail free of Tile's
drain + EVSEM barrier (~9-17 us).
"""

import sys

sys.path.insert(0, "/opt/trn_rl_repo")

import numpy as np

VOCAB = 49408
D_MODEL = 768
SEQ_LEN = 77
N_CORES = 8
D_SHARD = D_MODEL // N_CORES  # 96
P = 128  # SBUF partitions

_cache = {}


def build_nc(n=1):
    """The device program, unrolled n times serialized by semaphores
    (n=1 is the real kernel; n>1 is used by test.py's delta timing)."""
    from concourse import bass, mybir

    D = D_SHARD
    nc = bass.Bass(num_devices=N_CORES)
    inp_d = nc.dram_tensor(
        "inp", [P, D + 1], mybir.dt.float32, kind="ExternalInput"
    )
    idx_d = nc.dram_tensor(
        "idx", [P, 1], mybir.dt.float32, kind="ExternalInput"
    )
    tok_d = nc.dram_tensor(
        "tok_w", [VOCAB, D], mybir.dt.float32, kind="ExternalInput"
    )
    out_d = nc.dram_tensor(
        "out", [SEQ_LEN, D], mybir.dt.float32, kind="ExternalOutput"
    )

    with (
        nc.sbuf_tensor([P, 1], mybir.dt.float32) as idx_t,
        nc.sbuf_tensor([P, D], mybir.dt.float32) as tok_t,
        nc.semaphore() as s_in,
        nc.semaphore() as s_pos,
        nc.semaphore() as s_tok,
        nc.semaphore() as s_out,
        nc.Block() as block,
    ):

        @block.sync
        def _(sync):
            for i in range(n):
                sync.wait_ge(s_out, 16 * i)
                sync.dma_start(
                    out=idx_t[:SEQ_LEN, :], in_=idx_d[:SEQ_LEN, :]
                ).then_inc(s_in, 16)
                sync.dma_start(
                    out=out_d[:], in_=inp_d[:SEQ_LEN, :D]
                ).then_inc(s_pos, 16)
            sync.wait_ge(s_out, 16 * n)

        @block.gpsimd
        def _(gp):
            for i in range(n):
                gp.wait_ge(s_in, 16 * (i + 1))
                nc.gpsimd.indirect_dma_start(
                    out=tok_t[:SEQ_LEN, :],
                    out_offset=None,
                    in_=tok_d[:],
                    in_offset=bass.IndirectOffsetOnAxis(
                        ap=idx_t[:SEQ_LEN, :].bitcast(mybir.dt.int32),
                        axis=0,
                    ),
                    compute_op=mybir.AluOpType.bypass,
                ).then_inc(s_tok, 16)
                gp.wait_ge(s_pos, 16 * (i + 1))
                gp.dma_start(
                    out=out_d[:],
                    in_=tok_t[:SEQ_LEN, :],
                    accum_op=mybir.AluOpType.add,
                    single_packet=True,
                ).then_inc(s_out, 16)
            gp.wait_ge(s_out, 16 * n)

    return nc


def _get_runner():
    """Build the Bass module once and cache a jitted SPMD callable so
    repeat kernel() calls skip retracing/compilation."""
    if "runner" in _cache:
        return _cache["runner"]

    import jax
    from jax.sharding import Mesh, PartitionSpec
    from jax.experimental.shard_map import shard_map
    from concourse import bass2jax, mybir

    bass2jax.install_neuronx_cc_hook()
    nc = build_nc(1)
    partition_name = nc.partition_id_tensor.name if nc.partition_id_tensor else None

    in_names, out_names, out_avals, zero_outs = [], [], [], []
    for alloc in nc.m.functions[0].allocations:
        if not isinstance(alloc, mybir.MemoryLocationSet):
            continue
        name = alloc.memorylocations[0].name
        if alloc.kind == "ExternalInput":
            if name != partition_name:
                in_names.append(name)
        elif alloc.kind == "ExternalOutput":
            out_names.append(name)
            shape = tuple(alloc.tensor_shape)
            dtype = mybir.dt.np(alloc.dtype)
            out_avals.append(jax.core.ShapedArray(shape, dtype))
            zero_outs.append(np.zeros(shape, dtype))

    n_params = len(in_names)
    n_outs = len(out_avals)
    all_in_names = list(in_names) + list(out_names)
    if partition_name is not None:
        all_in_names.append(partition_name)

    def _body(*args):
        operands = list(args)
        if partition_name is not None:
            operands.append(bass2jax.partition_id_tensor())
        outs = bass2jax._bass_exec_p.bind(
            *operands,
            out_avals=tuple(out_avals),
            in_names=tuple(all_in_names),
            out_names=tuple(out_names),
            lowering_input_output_aliases=(),
            sim_require_finite=True,
            sim_require_nnan=True,
            nc=nc,
        )
        return tuple(outs)

    devices = jax.devices()[:N_CORES]
    assert len(devices) == N_CORES, f"need {N_CORES} devices, got {len(devices)}"
    mesh = Mesh(np.asarray(devices), ("core",))
    sharded = jax.jit(
        shard_map(
            _body,
            mesh=mesh,
            in_specs=(PartitionSpec("core"),) * (n_params + n_outs),
            out_specs=(PartitionSpec("core"),) * n_outs,
            check_rep=False,
        ),
        donate_argnums=tuple(range(n_params, n_params + n_outs)),
        keep_unused=True,
    )

    def run(concat_in_by_name):
        concat_in = [concat_in_by_name[name] for name in in_names]
        concat_zeros = [
            np.zeros((N_CORES * z.shape[0], *z.shape[1:]), z.dtype)
            for z in zero_outs
        ]
        out_arrs = sharded(*concat_in, *concat_zeros)
        return [
            {
                name: np.asarray(out_arrs[i]).reshape(
                    N_CORES, *out_avals[i].shape
                )[c]
                for i, name in enumerate(out_names)
            }
            for c in range(N_CORES)
        ]

    runner = {
        "run": run,
        "mesh": mesh,
        "PartitionSpec": PartitionSpec,
        "device_put": jax.device_put,
    }
    _cache["runner"] = runner
    return runner


def _tok_shards_device(tok, runner):
    """Column-shard the token table and push it to the 8 devices once;
    reuse across calls when the same table is passed again."""
    import jax

    sample = tok[::1997, ::13]
    key = (
        id(tok),
        tok.shape,
        str(tok.dtype),
        hash(sample.tobytes()),
        hash(tok[0, :8].tobytes()),
        hash(tok[-1, -8:].tobytes()),
    )
    hit = _cache.get("tok_dev")
    if hit is not None and hit[0] == key:
        return hit[1]
    # [V, 768] -> [8, V, 96] -> concat layout [8*V, 96], one pass
    shards = np.ascontiguousarray(
        tok.reshape(VOCAB, N_CORES, D_SHARD).swapaxes(0, 1)
    ).reshape(N_CORES * VOCAB, D_SHARD)
    sh = jax.sharding.NamedSharding(
        runner["mesh"], runner["PartitionSpec"]("core")
    )
    dev = runner["device_put"](shards, sh)
    jax.block_until_ready(dev)
    _cache["tok_dev"] = (key, dev)
    return dev


def kernel(
    input_ids: np.ndarray,
    position_ids: np.ndarray,
    token_embedding_weight: np.ndarray,
    position_embedding_weight: np.ndarray,
) -> np.ndarray:
    runner = _get_runner()

    ids = np.asarray(input_ids).astype(np.int32, copy=False)
    assert ids.shape == (SEQ_LEN,), ids.shape
    tok = np.ascontiguousarray(
        np.asarray(token_embedding_weight, dtype=np.float32)
    )
    pos_table = np.asarray(position_embedding_weight, dtype=np.float32)
    pids = np.asarray(position_ids).astype(np.int64, copy=False)
    if np.array_equal(pids, np.arange(SEQ_LEN)):
        pos = pos_table
    else:
        # CLIP always uses arange positions; reorder the tiny replicated
        # table during input prep if a caller ever passes something else.
        pos = pos_table[pids]

    # packed per-core input: [8, 128, 97] -> concat layout [8*128, 97]
    packed = np.zeros((N_CORES, P, D_SHARD + 1), np.float32)
    packed[:, :SEQ_LEN, :D_SHARD] = (
        pos.reshape(SEQ_LEN, N_CORES, D_SHARD).swapaxes(0, 1)
    )
    packed[:, :SEQ_LEN, D_SHARD] = ids.view(np.float32)[None, :]
    idx = np.zeros((N_CORES, P, 1), np.float32)
    idx[:, :SEQ_LEN, 0] = ids.view(np.float32)[None, :]

    concat_in = {
        "inp": packed.reshape(N_CORES * P, D_SHARD + 1),
        "idx": idx.reshape(N_CORES * P, 1),
        "tok_w": _tok_shards_device(tok, runner),
    }
    results = runner["run"](concat_in)
    out = np.concatenate([results[c]["out"] for c in range(N_CORES)], axis=1)
    return out[None, :, :]


# revision 7
# speedup vs baseline: 1.1116x; 1.0204x over previous
"""CLIP text embeddings (token-id gather + position-embedding add) on 8
Trainium2 NeuronCores via a raw Bass kernel.

Sharding: the 768-wide d_model axis is split 8 ways (96 columns per
core); the 77 token ids are replicated. Each core holds the full
49408-row token table restricted to its 96 columns, so every output
element is owned by exactly one core and no collective is needed (a
Megatron-style vocab-parallel split would pay a 236 KB AllReduce that
costs more than this whole kernel).

Per-core device program ("qmin2": 2 DMA-completion semaphores total):

  1. sync engine (HWDGE): DMA 1 loads ONLY the 308-byte id column into
     SBUF (one id per partition, bitcast to f32; the DMA legs are
     transfer-bound at ~30 B/ns, so not moving the 29.6 KB pos slice
     through SBUF saves ~1 us); DMA 2 copies the pos slice HBM->HBM
     straight into the output buffer, off the critical path.
  2. gpsimd: one indirect DMA gathers the 77 requested token rows from
     HBM into SBUF (bypass), then -- with NO completion wait -- a store
     with accum_op=add (single_packet) is enqueued on the SAME SWDGE
     queue (qPoolDynamic). Per-queue in-order descriptor execution
     makes the gather->store data dependency safe without a semaphore,
     and the CCE read-modify-write adds the gathered rows onto the pos
     values already sitting in the output.

Eliminating the gather-completion semaphore (~0.9 us DMA-sem
propagation), the separate vector add, and the pos bytes in the load
is worth ~2 us over the staged baseline: ~5.8 us vs ~7.9 us.

The host concatenates the 8 column shards into [1, 77, 768].

Raw Bass (no TileContext) keeps the kernel tail free of Tile's
drain + EVSEM barrier (~9-17 us).
"""

import sys

sys.path.insert(0, "/opt/trn_rl_repo")

import numpy as np

VOCAB = 49408
D_MODEL = 768
SEQ_LEN = 77
N_CORES = 8
D_SHARD = D_MODEL // N_CORES  # 96
P = 128  # SBUF partitions

_cache = {}


def build_nc(n=1):
    """The device program, unrolled n times serialized by semaphores
    (n=1 is the real kernel; n>1 is used by test.py's delta timing)."""
    from concourse import bass, mybir

    D = D_SHARD
    nc = bass.Bass(num_devices=N_CORES)
    inp_d = nc.dram_tensor(
        "inp", [P, D + 1], mybir.dt.float32, kind="ExternalInput"
    )
    idx_d = nc.dram_tensor(
        "idx", [P, 1], mybir.dt.float32, kind="ExternalInput"
    )
    tok_d = nc.dram_tensor(
        "tok_w", [VOCAB, D], mybir.dt.float32, kind="ExternalInput"
    )
    out_d = nc.dram_tensor(
        "out", [SEQ_LEN, D], mybir.dt.float32, kind="ExternalOutput"
    )

    with (
        nc.sbuf_tensor([P, 1], mybir.dt.float32) as idx_t,
        nc.sbuf_tensor([P, D], mybir.dt.float32) as tok_t,
        nc.semaphore() as s_in,
        nc.semaphore() as s_pos,
        nc.semaphore() as s_tok,
        nc.semaphore() as s_out,
        nc.Block() as block,
    ):

        @block.sync
        def _(sync):
            for i in range(n):
                sync.wait_ge(s_out, 16 * i)
                sync.dma_start(
                    out=idx_t[:SEQ_LEN, :], in_=idx_d[:SEQ_LEN, :]
                ).then_inc(s_in, 16)
                sync.dma_start(
                    out=out_d[:], in_=inp_d[:SEQ_LEN, :D]
                ).then_inc(s_pos, 16)
            sync.wait_ge(s_out, 16 * n)

        @block.gpsimd
        def _(gp):
            for i in range(n):
                gp.wait_ge(s_in, 16 * (i + 1))
                nc.gpsimd.indirect_dma_start(
                    out=tok_t[:SEQ_LEN, :],
                    out_offset=None,
                    in_=tok_d[:],
                    in_offset=bass.IndirectOffsetOnAxis(
                        ap=idx_t[:SEQ_LEN, :].bitcast(mybir.dt.int32),
                        axis=0,
                    ),
                    compute_op=mybir.AluOpType.bypass,
                ).then_inc(s_tok, 16)
                gp.wait_ge(s_pos, 16 * (i + 1))
                gp.dma_start(
                    out=out_d[:],
                    in_=tok_t[:SEQ_LEN, :],
                    accum_op=mybir.AluOpType.add,
                    single_packet=True,
                ).then_inc(s_out, 16)
            gp.wait_ge(s_out, 16 * n)

    return nc


def _get_runner():
    """Build the Bass module once and cache a jitted SPMD callable so
    repeat kernel() calls skip retracing/compilation."""
    if "runner" in _cache:
        return _cache["runner"]

    import jax
    from jax.sharding import Mesh, PartitionSpec
    from jax.experimental.shard_map import shard_map
    from concourse import bass2jax, mybir

    bass2jax.install_neuronx_cc_hook()
    nc = build_nc(1)
    partition_name = nc.partition_id_tensor.name if nc.partition_id_tensor else None

    in_names, out_names, out_avals, zero_outs = [], [], [], []
    for alloc in nc.m.functions[0].allocations:
        if not isinstance(alloc, mybir.MemoryLocationSet):
            continue
        name = alloc.memorylocations[0].name
        if alloc.kind == "ExternalInput":
            if name != partition_name:
                in_names.append(name)
        elif alloc.kind == "ExternalOutput":
            out_names.append(name)
            shape = tuple(alloc.tensor_shape)
            dtype = mybir.dt.np(alloc.dtype)
            out_avals.append(jax.core.ShapedArray(shape, dtype))
            zero_outs.append(np.zeros(shape, dtype))

    n_params = len(in_names)
    n_outs = len(out_avals)
    all_in_names = list(in_names) + list(out_names)
    if partition_name is not None:
        all_in_names.append(partition_name)

    def _body(*args):
        operands = list(args)
        if partition_name is not None:
            operands.append(bass2jax.partition_id_tensor())
        outs = bass2jax._bass_exec_p.bind(
            *operands,
            out_avals=tuple(out_avals),
            in_names=tuple(all_in_names),
            out_names=tuple(out_names),
            lowering_input_output_aliases=(),
            sim_require_finite=True,
            sim_require_nnan=True,
            nc=nc,
        )
        return tuple(outs)

    devices = jax.devices()[:N_CORES]
    assert len(devices) == N_CORES, f"need {N_CORES} devices, got {len(devices)}"
    mesh = Mesh(np.asarray(devices), ("core",))
    sharded = jax.jit(
        shard_map(
            _body,
            mesh=mesh,
            in_specs=(PartitionSpec("core"),) * (n_params + n_outs),
            out_specs=(PartitionSpec("core"),) * n_outs,
            check_rep=False,
        ),
        donate_argnums=tuple(range(n_params, n_params + n_outs)),
        keep_unused=True,
    )

    def run(concat_in_by_name):
        concat_in = [concat_in_by_name[name] for name in in_names]
        concat_zeros = [
            np.zeros((N_CORES * z.shape[0], *z.shape[1:]), z.dtype)
            for z in zero_outs
        ]
        out_arrs = sharded(*concat_in, *concat_zeros)
        return [
            {
                name: np.asarray(out_arrs[i]).reshape(
                    N_CORES, *out_avals[i].shape
                )[c]
                for i, name in enumerate(out_names)
            }
            for c in range(N_CORES)
        ]

    runner = {
        "run": run,
        "mesh": mesh,
        "PartitionSpec": PartitionSpec,
        "device_put": jax.device_put,
    }
    _cache["runner"] = runner
    return runner


def _tok_shards_device(tok, runner):
    """Column-shard the token table and push it to the 8 devices once;
    reuse across calls when the same table is passed again."""
    import jax

    sample = tok[::1997, ::13]
    key = (
        id(tok),
        tok.shape,
        str(tok.dtype),
        hash(sample.tobytes()),
        hash(tok[0, :8].tobytes()),
        hash(tok[-1, -8:].tobytes()),
    )
    hit = _cache.get("tok_dev")
    if hit is not None and hit[0] == key:
        return hit[1]
    # [V, 768] -> [8, V, 96] -> concat layout [8*V, 96], one pass
    shards = np.ascontiguousarray(
        tok.reshape(VOCAB, N_CORES, D_SHARD).swapaxes(0, 1)
    ).reshape(N_CORES * VOCAB, D_SHARD)
    sh = jax.sharding.NamedSharding(
        runner["mesh"], runner["PartitionSpec"]("core")
    )
    dev = runner["device_put"](shards, sh)
    jax.block_until_ready(dev)
    _cache["tok_dev"] = (key, dev)
    return dev


def kernel(
    input_ids: np.ndarray,
    position_ids: np.ndarray,
    token_embedding_weight: np.ndarray,
    position_embedding_weight: np.ndarray,
) -> np.ndarray:
    runner = _get_runner()

    ids = np.asarray(input_ids).astype(np.int32, copy=False)
    assert ids.shape == (SEQ_LEN,), ids.shape
    tok = np.ascontiguousarray(
        np.asarray(token_embedding_weight, dtype=np.float32)
    )
    pos_table = np.asarray(position_embedding_weight, dtype=np.float32)
    pids = np.asarray(position_ids).astype(np.int64, copy=False)
    if np.array_equal(pids, np.arange(SEQ_LEN)):
        pos = pos_table
    else:
        # CLIP always uses arange positions; reorder the tiny replicated
        # table during input prep if a caller ever passes something else.
        pos = pos_table[pids]

    # packed per-core input: [8, 128, 97] -> concat layout [8*128, 97]
    packed = np.zeros((N_CORES, P, D_SHARD + 1), np.float32)
    packed[:, :SEQ_LEN, :D_SHARD] = (
        pos.reshape(SEQ_LEN, N_CORES, D_SHARD).swapaxes(0, 1)
    )
    packed[:, :SEQ_LEN, D_SHARD] = ids.view(np.float32)[None, :]
    idx = np.zeros((N_CORES, P, 1), np.float32)
    idx[:, :SEQ_LEN, 0] = ids.view(np.float32)[None, :]

    concat_in = {
        "inp": packed.reshape(N_CORES * P, D_SHARD + 1),
        "idx": idx.reshape(N_CORES * P, 1),
        "tok_w": _tok_shards_device(tok, runner),
    }
    results = runner["run"](concat_in)
    out = np.concatenate([results[c]["out"] for c in range(N_CORES)], axis=1)
    return out[None, :, :]
